# revision 1
# baseline (speedup 1.0000x reference)
"""Trainium2 Bass kernel for nn_MultiHeadAttention (B=2, S=2048, H=1024, 16 heads).

Sharding: tensor-parallel over heads — 2 heads per core on 8 cores.
Each core computes Q/K/V projections for its 128 features (2 heads x 64),
scores + softmax + attention weights (written out), context, and a partial
output projection (row-parallel).  Host gathers attn heads and sums the
out-proj partials.

Self-contained: hardcodes shapes; only needs concourse + numpy + ml_dtypes.
"""
import numpy as np
import ml_dtypes

import concourse.bacc as bacc
import concourse.tile as tile
import concourse.mybir as mybir
from concourse.bass import ts
from concourse.bass_utils import run_bass_kernel_spmd
from concourse.masks import make_identity

B, S, HID = 2, 2048, 1024
NCORES, NH, DH = 8, 16, 64
HPC = NH // NCORES          # heads per core
F = HPC * DH                # feature slice per core
BS = B * S
NI = S // 128               # i-chunks per batch
NJ = S // 128               # j-chunks per batch
NC8 = HID // 128            # contraction chunks for projections

BF16 = mybir.dt.bfloat16
F32 = mybir.dt.float32
AF = mybir.ActivationFunctionType
BF = ml_dtypes.bfloat16


def build_nc():
    nc = bacc.Bacc("TRN2", target_bir_lowering=False, debug=False,
                   num_devices=NCORES)
    xT = nc.dram_tensor("xT", [HID, BS], BF16, kind="ExternalInput")
    wqT = nc.dram_tensor("wqT", [HID, F], BF16, kind="ExternalInput")
    wkT = nc.dram_tensor("wkT", [HID, F], BF16, kind="ExternalInput")
    wvT = nc.dram_tensor("wvT", [HID, F], BF16, kind="ExternalInput")
    woT = nc.dram_tensor("woT", [F, HID], BF16, kind="ExternalInput")
    bq = nc.dram_tensor("bq", [F, 1], F32, kind="ExternalInput")
    bk = nc.dram_tensor("bk", [F, 1], F32, kind="ExternalInput")
    bv = nc.dram_tensor("bv", [F, 1], F32, kind="ExternalInput")
    attn_o = nc.dram_tensor("attn_o", [B, HPC, S, S], F32, kind="ExternalOutput")
    out_p = nc.dram_tensor("out_p", [BS, HID], F32, kind="ExternalOutput")

    with tile.TileContext(nc) as tc:
        with tc.tile_pool(name="consts", bufs=1) as consts, \
             tc.tile_pool(name="psA", bufs=2, space="PSUM") as psA, \
             tc.tile_pool(name="psB", bufs=2, space="PSUM") as psB, \
             tc.tile_pool(name="psT", bufs=2, space="PSUM") as psT, \
             tc.tile_pool(name="sb_expS", bufs=3) as sb_expS, \
             tc.tile_pool(name="sb_attn", bufs=3) as sb_attn, \
             tc.tile_pool(name="sb_st", bufs=4) as sb_st, \
             tc.tile_pool(name="sb_out", bufs=2) as sb_out, \
             tc.tile_pool(name="sb_small", bufs=12) as sb_small:

            ident = consts.tile([128, 128], BF16)
            make_identity(nc, ident[:])

            # ---- load inputs ----
            xT_sb = consts.tile([128, NC8, BS], BF16)
            nc.sync.dma_start(out=xT_sb[:],
                              in_=xT[:, :].rearrange("(c p) i -> p c i", p=128))
            wq_sb = consts.tile([128, NC8, F], BF16)
            nc.sync.dma_start(out=wq_sb[:],
                              in_=wqT[:, :].rearrange("(c p) f -> p c f", p=128))
            wk_sb = consts.tile([128, NC8, F], BF16)
            nc.sync.dma_start(out=wk_sb[:],
                              in_=wkT[:, :].rearrange("(c p) f -> p c f", p=128))
            wv_sb = consts.tile([128, NC8, F], BF16)
            nc.sync.dma_start(out=wv_sb[:],
                              in_=wvT[:, :].rearrange("(c p) f -> p c f", p=128))
            wo_sb = consts.tile([128, HID], BF16)
            nc.sync.dma_start(out=wo_sb[:], in_=woT[:, :])
            bq_sb = consts.tile([128, 1], F32)
            nc.sync.dma_start(out=bq_sb[:], in_=bq[:, :])
            bk_sb = consts.tile([128, 1], F32)
            nc.sync.dma_start(out=bk_sb[:], in_=bk[:, :])
            bv_sb = consts.tile([128, 1], F32)
            nc.sync.dma_start(out=bv_sb[:], in_=bv[:, :])

            # ---- projections: qT/kT/vT [f=128, i=4096] (bf16, bias added) ----
            qT_sb = consts.tile([128, BS], BF16)
            kT_sb = consts.tile([128, BS], BF16)
            vT_sb = consts.tile([128, BS], BF16)
            for w_sb, b_sb, dest in ((wq_sb, bq_sb, qT_sb),
                                     (wk_sb, bk_sb, kT_sb),
                                     (wv_sb, bv_sb, vT_sb)):
                for ib in range(BS // 512):
                    ps = psB.tile([128, 512], F32, tag="b")
                    for cc in range(NC8):
                        nc.tensor.matmul(ps[:],
                                         lhsT=w_sb[:, cc, :],
                                         rhs=xT_sb[:, cc, ts(ib, 512)],
                                         start=(cc == 0), stop=(cc == NC8 - 1))
                    nc.scalar.activation(out=dest[:, ts(ib, 512)], in_=ps[:],
                                         func=AF.Identity, bias=b_sb[:], scale=1.0)

            # ---- V into [j, f] layout via PE transpose ----
            v_all = consts.tile([128, B, NJ, F], BF16)
            for b in range(B):
                for jc in range(NJ):
                    pt = psT.tile([128, 128], BF16, tag="t")
                    nc.tensor.transpose(pt[:], vT_sb[:, b * S + jc * 128:
                                                     b * S + (jc + 1) * 128], ident[:])
                    nc.scalar.copy(v_all[:, b, jc, :], pt[:])

            ctx_all = consts.tile([128, B, NI, F], BF16)

            # ---- main loop ----
            for b in range(B):
                for ic in range(NI):
                    i0 = b * S + ic * 128
                    for lh in range(HPC):
                        d0 = lh * DH
                        expS = sb_expS.tile([128, S], BF16, tag="e")
                        sume = sb_small.tile([128, 2], F32, tag="s2")
                        for jh in range(2):
                            ps = psA.tile([128, 1024], F32, tag="a")
                            for jq in range(2):
                                j0 = b * S + jh * 1024 + jq * 512
                                nc.tensor.matmul(
                                    ps[:, ts(jq, 512)],
                                    lhsT=qT_sb[d0:d0 + DH, i0:i0 + 128],
                                    rhs=kT_sb[d0:d0 + DH, j0:j0 + 512],
                                    start=True, stop=True)
                            nc.scalar.activation(out=expS[:, ts(jh, 1024)], in_=ps[:],
                                                 func=AF.Exp,
                                                 accum_out=sume[:, jh:jh + 1])
                        sumx = sb_small.tile([128, 1], F32, tag="s1")
                        nc.vector.tensor_add(sumx[:], sume[:, 0:1], sume[:, 1:2])
                        recip = sb_small.tile([128, 1], F32, tag="r1")
                        nc.vector.reciprocal(recip[:], sumx[:])

                        attn_t = sb_attn.tile([128, S], F32, tag="at")
                        nc.vector.tensor_scalar_mul(attn_t[:, 0:1024],
                                                    expS[:, 0:1024], recip[:])
                        nc.scalar.activation(out=attn_t[:, 1024:2048],
                                             in_=expS[:, 1024:2048],
                                             func=AF.Copy, bias=0.0, scale=recip[:])
                        nc.sync.dma_start(
                            out=attn_o[b, lh, ic * 128:(ic + 1) * 128, :],
                            in_=attn_t[:])

                        psc = psB.tile([128, DH], F32, tag="b")
                        for jc in range(NJ):
                            pt = psT.tile([128, 128], BF16, tag="t")
                            nc.tensor.transpose(pt[:], expS[:, ts(jc, 128)], ident[:])
                            st = sb_st.tile([128, 128], BF16, tag="st")
                            if jc % 4 == 3:
                                nc.scalar.copy(st[:], pt[:])
                            else:
                                nc.vector.tensor_copy(st[:], pt[:])
                            nc.tensor.matmul(psc[:], lhsT=st[:],
                                             rhs=v_all[:, b, jc, d0:d0 + DH],
                                             start=(jc == 0), stop=(jc == NJ - 1))
                        nc.vector.tensor_scalar_mul(
                            ctx_all[:, b, ic, d0:d0 + DH], psc[:], recip[:])

                    # ---- out-proj for this (b, ic) row block ----
                    pt = psT.tile([128, 128], BF16, tag="t")
                    nc.tensor.transpose(pt[:], ctx_all[:, b, ic, :], ident[:])
                    ctxT = sb_st.tile([128, 128], BF16, tag="st")
                    nc.vector.tensor_copy(ctxT[:], pt[:])
                    po = psA.tile([128, 1024], F32, tag="a")
                    for oh in range(2):
                        nc.tensor.matmul(po[:, ts(oh, 512)], lhsT=ctxT[:],
                                         rhs=wo_sb[:, ts(oh, 512)],
                                         start=True, stop=True)
                    ot = sb_out.tile([128, 1024], F32, tag="o")
                    nc.scalar.copy(ot[:], po[:])
                    nc.sync.dma_start(out=out_p[i0:i0 + 128, :], in_=ot[:])

    nc.finalize()
    return nc


_NC = None


def _get_nc():
    global _NC
    if _NC is None:
        _NC = build_nc()
    return _NC


def kernel(query, Wq, bq, Wk, bk, Wv, bv, Wo, bo):
    query = np.asarray(query, dtype=np.float32)
    Wq = np.asarray(Wq, dtype=np.float32)
    Wk = np.asarray(Wk, dtype=np.float32)
    Wv = np.asarray(Wv, dtype=np.float32)
    Wo = np.asarray(Wo, dtype=np.float32)
    bq = np.asarray(bq, dtype=np.float32)
    bk = np.asarray(bk, dtype=np.float32)
    bv = np.asarray(bv, dtype=np.float32)
    bo = np.asarray(bo, dtype=np.float32)

    x2d = query.reshape(BS, HID)
    xT_bf = np.ascontiguousarray(x2d.T).astype(BF)

    scale = 1.0 / np.sqrt(np.float32(DH))
    in_maps = []
    for c in range(NCORES):
        rows = slice(c * F, (c + 1) * F)
        in_maps.append({
            "xT": xT_bf,
            "wqT": np.ascontiguousarray((Wq[rows] * scale).T).astype(BF),
            "wkT": np.ascontiguousarray(Wk[rows].T).astype(BF),
            "wvT": np.ascontiguousarray(Wv[rows].T).astype(BF),
            "woT": np.ascontiguousarray(Wo[:, rows].T).astype(BF),
            "bq": (bq[rows] * scale).reshape(F, 1).astype(np.float32),
            "bk": bk[rows].reshape(F, 1).astype(np.float32),
            "bv": bv[rows].reshape(F, 1).astype(np.float32),
        })

    nc = _get_nc()
    res = run_bass_kernel_spmd(nc, in_maps, core_ids=list(range(NCORES)))

    attn = np.concatenate([res.results[c]["attn_o"] for c in range(NCORES)],
                          axis=1)
    out = res.results[0]["out_p"].astype(np.float64)
    for c in range(1, NCORES):
        out += res.results[c]["out_p"]
    out = (out + bo).astype(np.float32).reshape(B, S, HID)
    return out, attn
